# revision 6
# baseline (speedup 1.0000x reference)
"""Dice coefficient metric kernel for TRN2 (8 NeuronCores, SPMD batch-parallel).

Reference computation (all fp32):
    inter[b,c] = sum_hw prd*tgt
    union[b,c] = sum_hw prd + sum_hw tgt + EPS
    dice[b,c]  = (2*inter + EPS) / union
    out[c]     = mean_b dice[b,c]

Sharding: batch dim (16) split across 8 cores -> 2 batches (8 (b,c) slabs)
per core.  Each slab (1024x1024 fp32, 4 MiB) is streamed HBM->SBUF as a
[128, 8192] tile and reduced on the DVE with two fused tensor_tensor_reduce
ops (product+accum for inter, sum+accum for union).  Per-partition partials
land in a [128, 16] stats tile; a ones-vector matmul on the TensorEngine
collapses the partition dim.  The per-core dice partial (4 floats) is
all-reduced across the 8 cores and scaled by 1/B on device.
"""

import numpy as np

import concourse.bass as bass
import concourse.tile as tile
from concourse import bacc, mybir
from concourse.bass_utils import run_bass_kernel_spmd

B, C, H, W = 16, 4, 1024, 1024
N_CORES = 8
P = 128
EPS = 1e-6

B_LOC = B // N_CORES          # batches per core
SLABS = B_LOC * C             # (b,c) slabs per core
F = (H * W) // P              # free dim per slab tile

# When True the batch-mean finishes on device via an AllReduce; when False
# each core emits its per-core dice sum and the host averages.
USE_COLLECTIVE = True


def _build_nc(slabs: int, feat: int, c: int, n_cores: int):
    """Build + compile the per-core Bass program (same program on all cores)."""
    nc = bacc.Bacc(
        "TRN2", target_bir_lowering=False, debug=False, num_devices=n_cores
    )
    f32 = mybir.dt.float32
    prd = nc.dram_tensor("prd", [slabs, P, feat], f32, kind="ExternalInput")
    tgt = nc.dram_tensor("tgt", [slabs, P, feat], f32, kind="ExternalInput")
    out = nc.dram_tensor("out", [1, c], f32, kind="ExternalOutput")

    add = mybir.AluOpType.add
    mult = mybir.AluOpType.mult

    with tile.TileContext(nc) as tc:
        with (
            tc.tile_pool(name="io", bufs=2) as io_pool,
            tc.tile_pool(name="work", bufs=1) as work_pool,
            tc.tile_pool(name="psum", bufs=1, space=bass.MemorySpace.PSUM) as psum_pool,
            tc.tile_pool(name="dram", bufs=1, space=bass.MemorySpace.DRAM) as dram_pool,
        ):
            # Per-partition reduction partials: cols [0:slabs] = inter,
            # cols [slabs:2*slabs] = union (sum of prd+tgt).
            stats = work_pool.tile([P, 2 * slabs], f32)
            scratch = work_pool.tile([P, feat], f32)

            for s in range(slabs):
                pt = io_pool.tile([P, feat], f32, tag="prd")
                nc.gpsimd.dma_start(pt[:], prd[s])
                tt = io_pool.tile([P, feat], f32, tag="tgt")
                nc.gpsimd.dma_start(tt[:], tgt[s])

                # accum_out = sum((pt * 1) op1 tt): op1=mult -> inter partial,
                # op1=add -> union partial.  (tensor_tensor_reduce crashes the
                # exec unit on this runtime; scalar_tensor_tensor doesn't.)
                nc.vector.scalar_tensor_tensor(
                    out=scratch[:], in0=pt[:], scalar=1.0, in1=tt[:],
                    op0=mult, op1=mult,
                    accum_out=stats[:, s : s + 1],
                )
                nc.vector.scalar_tensor_tensor(
                    out=scratch[:], in0=pt[:], scalar=1.0, in1=tt[:],
                    op0=mult, op1=add,
                    accum_out=stats[:, slabs + s : slabs + s + 1],
                )

            # Collapse the 128 partitions: red[0, :] = ones.T @ stats.
            ones = work_pool.tile([P, 1], f32)
            nc.vector.memset(ones[:], 1.0)
            ps = psum_pool.tile([1, 2 * slabs], f32)
            nc.tensor.matmul(ps[:], ones[:], stats[:], start=True, stop=True)

            red = work_pool.tile([1, 2 * slabs], f32)
            nc.vector.tensor_copy(red[:], ps[:])
            inter = red[0:1, 0:slabs]
            usum = red[0:1, slabs : 2 * slabs]

            fin = work_pool.tile([1, 2 * slabs + c], f32)
            num = fin[0:1, 0:slabs]            # 2*inter + EPS
            den = fin[0:1, slabs : 2 * slabs]  # union = usum + EPS
            nc.vector.tensor_scalar(num, inter, 2.0, EPS, mult, add)
            nc.vector.tensor_scalar(den, usum, EPS, None, add)
            rec = work_pool.tile([1, slabs], f32)
            nc.vector.reciprocal(rec[:], den)
            dice = work_pool.tile([1, slabs], f32)
            nc.vector.tensor_mul(dice[:], num, rec[:])

            # Sum this core's B_LOC batches per channel (slab s = b*c_dim + ch),
            # then all-reduce over cores and scale by 1/B.
            part = fin[0:1, 2 * slabs : 2 * slabs + c]
            nc.vector.tensor_add(part, dice[0:1, 0:c], dice[0:1, c : 2 * c])

            if USE_COLLECTIVE:
                cc_in = dram_pool.tile([1, c], f32)
                cc_out = dram_pool.tile([1, c], f32)
                nc.gpsimd.dma_start(cc_in[:], part)
                nc.gpsimd.collective_compute(
                    "AllReduce",
                    add,
                    replica_groups=[list(range(n_cores))],
                    ins=[cc_in.opt()],
                    outs=[cc_out.opt()],
                )
                res = work_pool.tile([1, c], f32)
                nc.gpsimd.dma_start(res[:], cc_out[:])
                nc.vector.tensor_scalar_mul(
                    res[:], res[:], 1.0 / (B_LOC * n_cores)
                )
                nc.gpsimd.dma_start(out[0:1, :], res[:])
            else:
                nc.gpsimd.dma_start(out[0:1, :], part)

    nc.compile()
    return nc


_NC_CACHE: dict = {}


def _get_nc():
    key = (SLABS, F, C, N_CORES)
    if key not in _NC_CACHE:
        _NC_CACHE[key] = _build_nc(*key)
    return _NC_CACHE[key]


def _shard_inputs(prd: np.ndarray, tgt: np.ndarray):
    in_maps = []
    for i in range(N_CORES):
        sl = slice(i * B_LOC, (i + 1) * B_LOC)
        in_maps.append(
            {
                "prd": np.ascontiguousarray(prd[sl]).reshape(SLABS, P, F),
                "tgt": np.ascontiguousarray(tgt[sl]).reshape(SLABS, P, F),
            }
        )
    return in_maps


def kernel(prd: np.ndarray, tgt: np.ndarray, _trace: bool = False):
    prd = np.asarray(prd, dtype=np.float32)
    tgt = np.asarray(tgt, dtype=np.float32)
    assert prd.shape == (B, C, H, W) and tgt.shape == (B, C, H, W)

    nc = _get_nc()
    in_maps = _shard_inputs(prd, tgt)
    res = run_bass_kernel_spmd(nc, in_maps, list(range(N_CORES)), trace=_trace)
    if USE_COLLECTIVE:
        out = res.results[0]["out"].reshape(C).astype(np.float32)
    else:
        out = (
            sum(r["out"].reshape(C).astype(np.float64) for r in res.results) / B
        ).astype(np.float32)
    if _trace:
        return out, res
    return out


# revision 7
# speedup vs baseline: 1.7126x; 1.7126x over previous
"""Dice coefficient metric kernel for TRN2 (8 NeuronCores, SPMD batch-parallel).

Reference computation (all fp32):
    inter[b,c] = sum_hw prd*tgt
    union[b,c] = sum_hw prd + sum_hw tgt + EPS
    dice[b,c]  = (2*inter + EPS) / union
    out[c]     = mean_b dice[b,c]

Sharding: batch dim (16) split across 8 cores -> 2 batches (8 (b,c) slabs)
per core.  Each slab (1024x1024 fp32, 4 MiB) is streamed HBM->SBUF in two
[128, 4096] half-slab tiles (prd on the SP HWDGE ring, tgt on the ACT ring)
and reduced on the DVE with two fused scalar_tensor_tensor ops per half
(product+accum for inter, sum+accum for union).  Per-partition partials land
in a [128, 4*slabs] stats tile; one ones-vector matmul on the TensorEngine
collapses the partition dim, tiny vector ops combine the halves and form
dice, and each core DMAs its per-core dice partial (4 floats) out.  The
host sums the 8 partials and divides by B (the batch mean) while gathering.

The device-side AllReduce variant (USE_COLLECTIVE=True) is kept for
reference but off by default: on this runtime a 16-byte 8-core AllReduce
measures ~98us of fixed latency (vs ~210us total kernel), and HWDGE DMA
deadlocks when a collective is present in the NEFF, forcing slower SWDGE
loads on top.
"""

import numpy as np

import concourse.bass as bass
import concourse.tile as tile
from concourse import bacc, mybir
from concourse.bass_utils import run_bass_kernel_spmd

B, C, H, W = 16, 4, 1024, 1024
N_CORES = 8
P = 128
EPS = 1e-6

B_LOC = B // N_CORES          # batches per core
SLABS = B_LOC * C             # (b,c) slabs per core
F = (H * W) // P              # free dim per full slab
HALF = F // 2                 # free dim per loaded tile

USE_COLLECTIVE = False


def _build_nc(slabs: int, feat: int, c: int, n_cores: int):
    """Build + compile the per-core Bass program (same program on all cores)."""
    nc = bacc.Bacc(
        "TRN2", target_bir_lowering=False, debug=False, num_devices=n_cores
    )
    f32 = mybir.dt.float32
    half = feat // 2
    prd = nc.dram_tensor("prd", [slabs, P, feat], f32, kind="ExternalInput")
    tgt = nc.dram_tensor("tgt", [slabs, P, feat], f32, kind="ExternalInput")
    out = nc.dram_tensor("out", [1, c], f32, kind="ExternalOutput")

    add = mybir.AluOpType.add
    mult = mybir.AluOpType.mult

    # Without a collective in the NEFF the HWDGE rings (sync/scalar) are
    # safe and faster than SWDGE; with one they deadlock -> use gpsimd.
    load_p = nc.gpsimd if USE_COLLECTIVE else nc.sync
    load_t = nc.gpsimd if USE_COLLECTIVE else nc.scalar

    with tile.TileContext(nc) as tc:
        with (
            tc.tile_pool(name="io", bufs=3) as io_pool,
            tc.tile_pool(name="work", bufs=1) as work_pool,
            tc.tile_pool(name="psum", bufs=1, space=bass.MemorySpace.PSUM) as psum_pool,
            tc.tile_pool(name="dram", bufs=1, space=bass.MemorySpace.DRAM) as dram_pool,
        ):
            # Per-partition partials, one column per (half, kind, slab):
            # [0:s]=inter_h0 [s:2s]=union_h0 [2s:3s]=inter_h1 [3s:4s]=union_h1
            stats = work_pool.tile([P, 4 * slabs], f32)
            scratch = work_pool.tile([P, half], f32)

            for s in range(slabs):
                for h in range(2):
                    pt = io_pool.tile([P, half], f32, tag="prd")
                    load_p.dma_start(pt[:], prd[s, :, h * half : (h + 1) * half])
                    tt = io_pool.tile([P, half], f32, tag="tgt")
                    load_t.dma_start(tt[:], tgt[s, :, h * half : (h + 1) * half])

                    # accum_out = sum((pt * 1) op1 tt): mult -> inter partial,
                    # add -> union partial.  (tensor_tensor_reduce crashes the
                    # exec unit on this runtime; scalar_tensor_tensor works.)
                    base = 2 * slabs * h
                    nc.vector.scalar_tensor_tensor(
                        out=scratch[:], in0=pt[:], scalar=1.0, in1=tt[:],
                        op0=mult, op1=mult,
                        accum_out=stats[:, base + s : base + s + 1],
                    )
                    nc.vector.scalar_tensor_tensor(
                        out=scratch[:], in0=pt[:], scalar=1.0, in1=tt[:],
                        op0=mult, op1=add,
                        accum_out=stats[:, base + slabs + s : base + slabs + s + 1],
                    )

            # Collapse the 128 partitions: red[0, :] = ones.T @ stats.
            ones = work_pool.tile([P, 1], f32)
            nc.vector.memset(ones[:], 1.0)
            ps = psum_pool.tile([1, 4 * slabs], f32)
            nc.tensor.matmul(ps[:], ones[:], stats[:], start=True, stop=True)

            red = work_pool.tile([1, 4 * slabs], f32)
            nc.vector.tensor_copy(red[:], ps[:])

            # Combine the two halves, then dice = (2*inter+EPS)/(union+EPS).
            fin = work_pool.tile([1, 2 * slabs + c], f32)
            inter = fin[0:1, 0:slabs]
            usum = fin[0:1, slabs : 2 * slabs]
            nc.vector.tensor_add(
                inter, red[0:1, 0:slabs], red[0:1, 2 * slabs : 3 * slabs]
            )
            nc.vector.tensor_add(
                usum, red[0:1, slabs : 2 * slabs], red[0:1, 3 * slabs : 4 * slabs]
            )
            num = work_pool.tile([1, slabs], f32)
            nc.vector.tensor_scalar(num[:], inter, 2.0, EPS, mult, add)
            den = work_pool.tile([1, slabs], f32)
            nc.vector.tensor_scalar(den[:], usum, EPS, None, add)
            rec = work_pool.tile([1, slabs], f32)
            nc.vector.reciprocal(rec[:], den[:])
            dice = work_pool.tile([1, slabs], f32)
            nc.vector.tensor_mul(dice[:], num[:], rec[:])

            # Per-core partial: sum of this core's B_LOC batches per channel
            # (slab s = b_local*C + ch).
            part = fin[0:1, 2 * slabs : 2 * slabs + c]
            nc.vector.tensor_add(part, dice[0:1, 0:c], dice[0:1, c : 2 * c])

            if USE_COLLECTIVE:
                cc_in = dram_pool.tile([1, c], f32)
                cc_out = dram_pool.tile([1, c], f32)
                nc.gpsimd.dma_start(cc_in[:], part)
                nc.gpsimd.collective_compute(
                    "AllReduce",
                    add,
                    replica_groups=[list(range(n_cores))],
                    ins=[cc_in.opt()],
                    outs=[cc_out.opt()],
                )
                res = work_pool.tile([1, c], f32)
                nc.gpsimd.dma_start(res[:], cc_out[:])
                nc.vector.tensor_scalar_mul(
                    res[:], res[:], 1.0 / (B_LOC * n_cores)
                )
                nc.gpsimd.dma_start(out[0:1, :], res[:])
            else:
                nc.sync.dma_start(out[0:1, :], part)

    nc.compile()
    return nc


_NC_CACHE: dict = {}


def _get_nc():
    key = (SLABS, F, C, N_CORES)
    if key not in _NC_CACHE:
        _NC_CACHE[key] = _build_nc(*key)
    return _NC_CACHE[key]


def _shard_inputs(prd: np.ndarray, tgt: np.ndarray):
    in_maps = []
    for i in range(N_CORES):
        sl = slice(i * B_LOC, (i + 1) * B_LOC)
        in_maps.append(
            {
                "prd": np.ascontiguousarray(prd[sl]).reshape(SLABS, P, F),
                "tgt": np.ascontiguousarray(tgt[sl]).reshape(SLABS, P, F),
            }
        )
    return in_maps


def kernel(prd: np.ndarray, tgt: np.ndarray, _trace: bool = False):
    prd = np.asarray(prd, dtype=np.float32)
    tgt = np.asarray(tgt, dtype=np.float32)
    assert prd.shape == (B, C, H, W) and tgt.shape == (B, C, H, W)

    nc = _get_nc()
    in_maps = _shard_inputs(prd, tgt)
    res = run_bass_kernel_spmd(nc, in_maps, list(range(N_CORES)), trace=_trace)
    if USE_COLLECTIVE:
        out = res.results[0]["out"].reshape(C).astype(np.float32)
    else:
        out = (
            sum(r["out"].reshape(C).astype(np.float64) for r in res.results) / B
        ).astype(np.float32)
    if _trace:
        return out, res
    return out
